# revision 1
# baseline (speedup 1.0000x reference)
"""CRF layer (dense CRF with Gaussian spatial kernel) on 8 TRN2 cores.

Per-core: row shard (H/8 rows) + 45-row halo, no inter-core comms.
State lives in B-layout [w-partitions, (class, h)] fp16.
Each iteration:
  pass1: W-blur as data-stationary banded matmuls (B -> A layout)
  pass2: H-blur likewise (A -> B), Potts scale & -unary folded in (PSUM)
  softmax: exp (ACT, from PSUM), sums (GPSIMD), recip+mult (DVE)
Normalization (1/sqrt(blur(ones))) is separable and baked into the band
matrices on the host.
"""
import numpy as np
from contextlib import ExitStack

import concourse.bass as bass
import concourse.mybir as mybir
import concourse.tile as tile
from concourse.vector_clock import ScopedClock, VectorClock

F16 = mybir.dt.float16
F32 = mybir.dt.float32
AF = mybir.ActivationFunctionType

# ---------------- problem constants ----------------
H = 2048
W = 2048
C = 4
SIGMA = 3.0
R = 9            # ceil(3*sigma)
ITERS = 5
NCORES = 8
SH = H // NCORES          # 256 rows per core
HALO = ITERS * R          # 45
HP = SH + 2 * HALO        # 346 rows incl halo
HPS = 384                 # padded to 3*128
NT = HPS // 128           # 3 h tiles
WT = W // 128             # 16 w tiles
WINP = 160                # padded band window (<=146 used)
SHIFT = 4.0               # logit shift for fp16-safe softmax

# ---------------- walrus compat (1 sync-wait per instruction) ----------------
_PATCHED = False


def _patch_drain():
    _orig = tile.TileContext._drain_and_barrier

    def _patched(self, tick_clock, wait_clock):
        gc = tick_clock.global_clock
        n = len(gc)
        for p in range(n):
            t = gc[p]
            if t > 0:
                vec = [0] * n
                vec[p] = t
                nop = self.nc.sync.nop()
                wait_clock.add_sem_waits(
                    nop.ins, ScopedClock({None: VectorClock(vec)})
                )
        full = ScopedClock({None: gc})
        for ec in wait_clock.engine_clocks:
            ec.update_past(full)
        _orig(self, tick_clock, wait_clock)

    tile.TileContext._drain_and_barrier = _patched


def install_compat():
    global _PATCHED
    if not _PATCHED:
        _patch_drain()
        _PATCHED = True


def split_multi_waits(nc):
    """Any instruction with >1 sync wait gets wait-only EventSemaphores
    inserted before it on the same engine (engines run in order)."""
    n_split = 0
    for fn in nc.m.functions:
        for bb in fn.blocks:
            insts = list(bb.instructions)
            out = []
            changed = False
            for inst in insts:
                si = inst.sync_info
                waits = list(si.on_wait) if si is not None else []
                if len(waits) > 1:
                    for j, w in enumerate(waits[:-1]):
                        es = mybir.InstEventSemaphore(
                            name=f"{inst.name}-esw{j}", ins=[], outs=[]
                        )
                        es.engine = inst.engine
                        es.sync_info = mybir.SyncInfo(on_wait=[w], on_update=[])
                        out.append(es)
                        n_split += 1
                    inst.sync_info = mybir.SyncInfo(
                        on_wait=[waits[-1]], on_update=list(si.on_update)
                    )
                    changed = True
                out.append(inst)
            if changed:
                bb.instructions = out
    return n_split


# ---------------- host-side band construction ----------------
def gauss_taps():
    x = np.arange(-R, R + 1, dtype=np.float64)
    return np.exp(-0.5 * (x / SIGMA) ** 2)


def norm_vec(n):
    k = gauss_taps()
    v = np.convolve(np.ones(n, dtype=np.float64), k, mode="same")
    return v


def w_windows():
    wins = []
    for t in range(WT):
        lo = max(0, 128 * t - R)
        hi = min(W, 128 * t + 128 + R)
        wins.append((lo, hi))
    return wins


def h_windows():
    wins = []
    for t in range(NT):
        lo = max(0, 128 * t - R)
        hi = min(HP, 128 * t + 128 + R)
        wins.append((lo, hi))
    return wins


def build_bw():
    """W-direction band blocks [WT, 128, WINP] fp16 (shared by all cores).
    bw[t, i, j] = nw[win] ... = nw[w_in]*k[w_in-w_out]*nw[w_out]."""
    k = gauss_taps()
    nw = 1.0 / np.sqrt(norm_vec(W))
    out = np.zeros((WT, 128, WINP), dtype=np.float64)
    for t, (lo, hi) in enumerate(w_windows()):
        for i in range(128):
            wi = 128 * t + i
            if wi >= W:
                continue
            for j in range(hi - lo):
                wo = lo + j
                d = wi - wo
                if -R <= d <= R:
                    out[t, i, j] = nw[wi] * k[d + R] * nw[wo]
    return out.astype(np.float16)


def build_bh(core, alphas):
    """H-direction band blocks [C, NT, 128, WINP] fp16, per core.
    Baked: per-class Potts scale (-alpha_c) and the global-row norm
    (zero at padded rows -> exact zero-pad behavior at shard edges)."""
    k = gauss_taps()
    vh = norm_vec(H)
    nh_g = 1.0 / np.sqrt(vh)
    g0 = core * SH - HALO
    nh = np.zeros(HPS, dtype=np.float64)
    for h in range(HP):
        g = g0 + h
        if 0 <= g < H:
            nh[h] = nh_g[g]
    base = np.zeros((NT, 128, WINP), dtype=np.float64)
    for t, (lo, hi) in enumerate(h_windows()):
        for i in range(128):
            hi_in = 128 * t + i
            if hi_in >= HPS:
                continue
            for j in range(hi - lo):
                ho = lo + j
                d = hi_in - ho
                if -R <= d <= R:
                    base[t, i, j] = nh[hi_in] * k[d + R] * nh[ho]
    out = np.zeros((C, NT, 128, WINP), dtype=np.float64)
    for c in range(C):
        out[c] = -alphas[c] * base
    return out.astype(np.float16)


def host_prep(unary, spatial_weights, compatibility_matrix):
    """Returns (in_maps, alphas). in_maps[core] keys: negu, bw, bh, ident."""
    M = np.asarray(spatial_weights, np.float64) @ np.asarray(
        compatibility_matrix, np.float64
    )
    offd = M - np.diag(np.diag(M))
    if np.abs(offd).max() > 1e-5 * max(np.abs(M).max(), 1e-30):
        raise NotImplementedError(
            "non-diagonal combined compatibility not supported"
        )
    alphas = np.diag(M).copy()

    bw = build_bw()
    ident = np.eye(128, dtype=np.float16)
    un_full = (-np.asarray(unary, np.float32) - SHIFT)  # [H, W, C]

    in_maps = []
    for core in range(NCORES):
        g0 = core * SH - HALO
        sl = np.zeros((HPS, W, C), dtype=np.float32)
        lo = max(0, g0)
        hi = min(H, g0 + HP)
        sl[lo - g0:hi - g0] = un_full[lo:hi]
        # [h, w, c] -> [w, c, h] -> [WT, 128, C, HPS]
        negu = (
            np.ascontiguousarray(sl.transpose(1, 2, 0))
            .astype(np.float16)
            .reshape(WT, 128, C, HPS)
        )
        in_maps.append(
            {
                "negu": negu,
                "bw": bw,
                "bh": build_bh(core, alphas),
                "ident": ident,
            }
        )
    return in_maps, alphas


def gather_output(results):
    """results[core]["qout"]: [WT, 128, C, SH] fp16 -> [H, W, C] fp32."""
    out = np.empty((H, W, C), dtype=np.float32)
    for core in range(NCORES):
        q = results[core]["qout"].astype(np.float32)  # [WT,128,C,SH]
        q = q.reshape(W, C, SH).transpose(2, 0, 1)    # [SH, W, C]
        out[core * SH:(core + 1) * SH] = q
    return out


# ---------------- device kernel ----------------
def seg_split(lo, hi, step=512):
    """Split [lo,hi) at multiples of step."""
    segs = []
    a = lo
    while a < hi:
        b = min(hi, (a // step + 1) * step)
        segs.append((a, b))
        a = b
    return segs


def build_nc(iters=ITERS, repeat=1):
    install_compat()
    nc = bass.Bass("TRN2", target_bir_lowering=False)
    negu_d = nc.dram_tensor("negu", [WT, 128, C, HPS], F16, kind="ExternalInput")
    bw_d = nc.dram_tensor("bw", [WT, 128, WINP], F16, kind="ExternalInput")
    bh_d = nc.dram_tensor("bh", [C, NT, 128, WINP], F16, kind="ExternalInput")
    id_d = nc.dram_tensor("ident", [128, 128], F16, kind="ExternalInput")
    qout_d = nc.dram_tensor("qout", [WT, 128, C, SH], F16, kind="ExternalOutput")

    wwins = w_windows()
    hwins = h_windows()

    with tile.TileContext(nc) as tc, ExitStack() as ctx:
        ctx.enter_context(
            nc.allow_low_precision(
                reason="softmax sums/recip in fp16 by design (shifted logits)"
            )
        )
        pers = ctx.enter_context(tc.tile_pool(name="pers", bufs=1))
        ps_pool = ctx.enter_context(tc.tile_pool(name="ps", bufs=2, space="PSUM"))
        scr = ctx.enter_context(tc.tile_pool(name="scr", bufs=3))
        outp = ctx.enter_context(tc.tile_pool(name="outp", bufs=3))

        negu = []
        qb = []
        for wt in range(WT):
            t = pers.tile([128, C, HPS], F16, tag=f"negu{wt}", name=f"negu{wt}")
            nc.sync.dma_start(t[:, :, :], negu_d[wt])
            negu.append(t)
            q = pers.tile([128, C, HPS], F16, tag=f"qb{wt}", name=f"qb{wt}")
            nc.vector.memset(q[:, :, HP:HPS], 0.0)
            qb.append(q)
        spa = [
            [
                pers.tile([128, W], F16, tag=f"spa{hc}_{c}", name=f"spa{hc}_{c}")
                for c in range(C)
            ]
            for hc in range(NT)
        ]
        bw = []
        for wt in range(WT):
            t = pers.tile([128, WINP], F16, tag=f"bw{wt}", name=f"bwt{wt}")
            nc.sync.dma_start(t[:, :], bw_d[wt])
            bw.append(t)
        bh = []
        for c in range(C):
            row = []
            for hc in range(NT):
                t = pers.tile([128, WINP], F16, tag=f"bh{c}_{hc}", name=f"bht{c}_{hc}")
                nc.sync.dma_start(t[:, :], bh_d[c, hc])
                row.append(t)
            bh.append(row)
        ident = pers.tile([128, 128], F16, tag="ident", name="ident")
        nc.sync.dma_start(ident[:, :], id_d[:, :])

        def softmax_block(wt, e_src_emit, last, vlo=0, vhi=HP):
            """e_src_emit(e_tile, vlo, vhi) emits exp instructions into e.
            Only rows [vlo, vhi) are computed (validity shrinks with the
            halo each iteration)."""
            n = vhi - vlo
            e = scr.tile([128, C, HP], F16, tag="e", name="e")
            e_src_emit(e, vlo, vhi)
            s2 = scr.tile([128, 2, HP], F16, tag="s2", name="s2")
            nc.gpsimd.tensor_add(
                s2[:, :, vlo:vhi], e[:, 0:2, vlo:vhi], e[:, 2:4, vlo:vhi]
            )
            s = scr.tile([128, HP], F16, tag="s", name="s")
            nc.gpsimd.tensor_add(s[:, vlo:vhi], s2[:, 0, vlo:vhi], s2[:, 1, vlo:vhi])
            r = scr.tile([128, HP], F16, tag="r", name="r")
            nc.vector.reciprocal(r[:, vlo:vhi], s[:, vlo:vhi])
            if not last:
                rb = r[:, vlo:vhi].unsqueeze(1).broadcast_to([128, C, n])
                nc.vector.tensor_tensor(
                    out=qb[wt][:, :, vlo:vhi], in0=e[:, :, vlo:vhi], in1=rb,
                    op=mybir.AluOpType.mult,
                )
            else:
                qo = outp.tile([128, C, SH], F16, tag="qo", name="qo")
                rb = r[:, HALO:HALO + SH].unsqueeze(1).broadcast_to([128, C, SH])
                nc.vector.tensor_tensor(
                    out=qo[:, :, :], in0=e[:, :, HALO:HALO + SH], in1=rb,
                    op=mybir.AluOpType.mult,
                )
                nc.sync.dma_start(qout_d[wt], qo[:, :, :])

        # ---- optional on-device repeat loop (benchmarking only) ----
        loop_cm = tc.For_i(0, repeat, 1) if repeat > 1 else None
        if loop_cm is not None:
            loop_cm.__enter__()

        # ---- init: Q0 = softmax(negu) ----
        for wt in range(WT):
            def emit_init(e, vlo, vhi, wt=wt):
                nc.scalar.activation(
                    e[:, 0:2, vlo:vhi], negu[wt][:, 0:2, vlo:vhi], AF.Exp
                )
                nc.scalar.activation(
                    e[:, 2:4, vlo:vhi], negu[wt][:, 2:4, vlo:vhi], AF.Exp
                )
            softmax_block(wt, emit_init, last=False)

        # ---- iterations ----
        for it in range(iters):
            last = it == iters - 1
            shrink = min(R * (it + 1), HALO)
            shrink -= shrink % 2  # keep slices 4B-aligned for DVE 2x modes
            vlo, vhi = shrink, HP - shrink
            # pass1: W-blur, B -> A. One 4-bank psum tile per (hc, c).
            for hc in range(NT):
                for c in range(C):
                    ps = ps_pool.tile([128, 4, 512], F32, tag="ps", name="ps")
                    mms = []
                    for wtile in range(WT):
                        lo, hi = wwins[wtile]
                        for (a, b) in seg_split(lo, hi):
                            mms.append((wtile, lo, a, b))
                    # start/stop are per 2KB PSUM bank
                    first_in_bank = [True] * 4
                    last_idx = {}
                    for idx, (wtile, lo, a, b) in enumerate(mms):
                        last_idx[a // 512] = idx
                    for idx, (wtile, lo, a, b) in enumerate(mms):
                        bank = a // 512
                        off = a % 512
                        nc.tensor.matmul(
                            ps[:, bank, off:off + b - a],
                            qb[wtile][:, c, 128 * hc:128 * (hc + 1)],
                            bw[wtile][:, a - lo:b - lo],
                            start=first_in_bank[bank],
                            stop=(last_idx[bank] == idx),
                        )
                        first_in_bank[bank] = False
                    if (hc * 4 + c) % 4 == 3:
                        nc.scalar.copy(spa[hc][c][:, 0:W], ps[:, :, :])
                    else:
                        nc.vector.tensor_copy(spa[hc][c][:, 0:W], ps[:, :, :])
            # pass2 + softmax, per w-tile. One 4-bank psum tile per wt.
            for wt in range(WT):
                ps = ps_pool.tile([128, 4, 512], F32, tag="ps", name="ps2")
                for c in range(C):
                    first = True
                    for hc in range(NT):
                        lo, hi = hwins[hc]
                        lo2, hi2 = max(lo, vlo), min(hi, vhi)
                        if lo2 >= hi2:
                            continue
                        nc.tensor.matmul(
                            ps[:, c, lo2:hi2],
                            spa[hc][c][:, 128 * wt:128 * (wt + 1)],
                            bh[c][hc][:, lo2 - lo:hi2 - lo],
                            start=first,
                            stop=False,
                        )
                        first = False
                    nc.tensor.matmul(
                        ps[:, c, vlo:vhi],
                        ident[:, :],
                        negu[wt][:, c, vlo:vhi],
                        start=False,
                        stop=True,
                    )

                def emit_blur(e, vl, vh, p=ps):
                    nc.scalar.activation(
                        e[:, :, vl:vh], p[:, :, vl:vh], AF.Exp
                    )
                softmax_block(wt, emit_blur, last=last, vlo=vlo, vhi=vhi)

        if loop_cm is not None:
            loop_cm.__exit__(None, None, None)

    split_multi_waits(nc)
    return nc


_NC_CACHE = None


def get_nc():
    global _NC_CACHE
    if _NC_CACHE is None:
        _NC_CACHE = build_nc()
    return _NC_CACHE


def kernel(unary, image, spatial_weights, compatibility_matrix):
    from concourse.bass_utils import run_bass_kernel_spmd

    in_maps, _ = host_prep(unary, spatial_weights, compatibility_matrix)
    nc = get_nc()
    res = run_bass_kernel_spmd(nc, in_maps, core_ids=list(range(NCORES)))
    return gather_output(res.results)



# revision 7
# speedup vs baseline: 1.1727x; 1.1727x over previous
"""CRF layer (dense CRF with Gaussian spatial kernel) on 8 TRN2 cores.

Per-core: row shard (H/8 rows) + 45-row halo, no inter-core comms.
State lives in B-layout [w-partitions, (class, h)] fp16.

Math restructuring vs the naive form (validated in fp64 to 3e-6):
- Equal Potts alphas + sum_c Q_c = 1  =>  only the differences
  D_c = Q_c - Q_3 (c<3) need blurring; class 3 needs no pairwise term
  (softmax shift invariance, shifting all logits by L_3).
- Shift-free softmax: e'_c = exp(L_c - L_3), e'_3 = 1, so
  s = 1 + sum_{c<3} e'_c,  D_c = (e'_c - 1) / s.  fp32 internals.

Each iteration:
  pass1: W-blur of D as data-stationary banded matmuls (B -> A layout)
  pass2: H-blur likewise (A -> B) + unary-diff via identity matmul (PSUM)
  softmax: exp (ACT, fp32), sums (Pool), reciprocal_approx_fast (DVE),
           D = (e-1)*r (DVE).  PSUM->SBUF copies cycle ACT/DVE/Pool.
Normalization (1/sqrt(blur(ones))) is separable and baked into the band
matrices on the host.
"""
import numpy as np
from contextlib import ExitStack

import concourse.bass as bass
import concourse.mybir as mybir
import concourse.tile as tile
from concourse.vector_clock import ScopedClock, VectorClock

F16 = mybir.dt.float16
F32 = mybir.dt.float32
AF = mybir.ActivationFunctionType
ALU = mybir.AluOpType

# ---------------- problem constants ----------------
H = 2048
W = 2048
C = 4
CM = 3           # blurred classes (differences vs class 3)
SIGMA = 3.0
R = 9            # ceil(3*sigma)
ITERS = 5
NCORES = 8
SH = H // NCORES          # 256 rows per core
HALO = ITERS * R          # 45
HP = SH + 2 * HALO        # 346 rows incl halo
HPS = 384                 # padded to 3*128
NT = HPS // 128           # 3 h tiles
WT = W // 128             # 16 w tiles
WINP = 160                # padded band window (<=146 used)

# ---------------- walrus compat (1 sync-wait per instruction) ----------------
_PATCHED = False


def _patch_drain():
    _orig = tile.TileContext._drain_and_barrier

    def _patched(self, tick_clock, wait_clock):
        gc = tick_clock.global_clock
        n = len(gc)
        for p in range(n):
            t = gc[p]
            if t > 0:
                vec = [0] * n
                vec[p] = t
                nop = self.nc.sync.nop()
                wait_clock.add_sem_waits(
                    nop.ins, ScopedClock({None: VectorClock(vec)})
                )
        full = ScopedClock({None: gc})
        for ec in wait_clock.engine_clocks:
            ec.update_past(full)
        _orig(self, tick_clock, wait_clock)

    tile.TileContext._drain_and_barrier = _patched


def install_compat():
    global _PATCHED
    if not _PATCHED:
        _patch_drain()
        _PATCHED = True


def split_multi_waits(nc):
    """Any instruction with >1 sync wait gets wait-only EventSemaphores
    inserted before it on the same engine (engines run in order)."""
    n_split = 0
    for fn in nc.m.functions:
        for bb in fn.blocks:
            insts = list(bb.instructions)
            out = []
            changed = False
            for inst in insts:
                si = inst.sync_info
                waits = list(si.on_wait) if si is not None else []
                if len(waits) > 1:
                    for j, w in enumerate(waits[:-1]):
                        es = mybir.InstEventSemaphore(
                            name=f"{inst.name}-esw{j}", ins=[], outs=[]
                        )
                        es.engine = inst.engine
                        es.sync_info = mybir.SyncInfo(on_wait=[w], on_update=[])
                        out.append(es)
                        n_split += 1
                    inst.sync_info = mybir.SyncInfo(
                        on_wait=[waits[-1]], on_update=list(si.on_update)
                    )
                    changed = True
                out.append(inst)
            if changed:
                bb.instructions = out
    return n_split


# ---------------- host-side band construction ----------------
def gauss_taps():
    x = np.arange(-R, R + 1, dtype=np.float64)
    return np.exp(-0.5 * (x / SIGMA) ** 2)


def norm_vec(n):
    k = gauss_taps()
    v = np.convolve(np.ones(n, dtype=np.float64), k, mode="same")
    return v


def w_windows():
    wins = []
    for t in range(WT):
        lo = max(0, 128 * t - R)
        hi = min(W, 128 * t + 128 + R)
        wins.append((lo, hi))
    return wins


def h_windows():
    wins = []
    for t in range(NT):
        lo = max(0, 128 * t - R)
        hi = min(HP, 128 * t + 128 + R)
        wins.append((lo, hi))
    return wins


def build_bw():
    """W-direction band blocks [WT, 128, WINP] fp16 (shared by all cores).
    bw[t, i, j] = nw[w_in]*k[w_in-w_out]*nw[w_out]."""
    k = gauss_taps()
    nw = 1.0 / np.sqrt(norm_vec(W))
    out = np.zeros((WT, 128, WINP), dtype=np.float64)
    for t, (lo, hi) in enumerate(w_windows()):
        for i in range(128):
            wi = 128 * t + i
            if wi >= W:
                continue
            for j in range(hi - lo):
                wo = lo + j
                d = wi - wo
                if -R <= d <= R:
                    out[t, i, j] = nw[wi] * k[d + R] * nw[wo]
    return out.astype(np.float16)


def build_bh(core, alpha):
    """H-direction band blocks [NT, 128, WINP] fp16, per core (shared by
    the 3 blurred classes).  Baked: Potts scale (-alpha) and the
    global-row norm (zero at padded rows -> exact zero-pad behavior at
    shard edges)."""
    k = gauss_taps()
    nh_g = 1.0 / np.sqrt(norm_vec(H))
    g0 = core * SH - HALO
    nh = np.zeros(HPS, dtype=np.float64)
    for h in range(HP):
        g = g0 + h
        if 0 <= g < H:
            nh[h] = nh_g[g]
    out = np.zeros((NT, 128, WINP), dtype=np.float64)
    for t, (lo, hi) in enumerate(h_windows()):
        for i in range(128):
            hi_in = 128 * t + i
            if hi_in >= HPS:
                continue
            for j in range(hi - lo):
                ho = lo + j
                d = hi_in - ho
                if -R <= d <= R:
                    out[t, i, j] = -alpha * nh[hi_in] * k[d + R] * nh[ho]
    return out.astype(np.float16)


def host_prep(unary, spatial_weights, compatibility_matrix):
    """Returns (in_maps, alpha). in_maps[core] keys: negu, bw, bh, ident."""
    M = np.asarray(spatial_weights, np.float64) @ np.asarray(
        compatibility_matrix, np.float64
    )
    offd = M - np.diag(np.diag(M))
    if np.abs(offd).max() > 1e-5 * max(np.abs(M).max(), 1e-30):
        raise NotImplementedError(
            "non-diagonal combined compatibility not supported"
        )
    alphas = np.diag(M).copy()
    if not np.allclose(alphas, alphas[0], rtol=1e-6, atol=1e-8):
        raise NotImplementedError("unequal Potts alphas not supported")
    alpha = float(alphas[0])

    bw = build_bw()
    ident = np.eye(128, dtype=np.float16)
    un = np.asarray(unary, np.float32)                  # [H, W, C]
    # L_c - L_3 unary part: u_3 - u_c  (c < 3)
    nprime = un[:, :, 3:4] - un[:, :, 0:3]              # [H, W, 3]

    in_maps = []
    for core in range(NCORES):
        g0 = core * SH - HALO
        sl = np.zeros((HPS, W, CM), dtype=np.float32)
        lo = max(0, g0)
        hi = min(H, g0 + HP)
        sl[lo - g0:hi - g0] = nprime[lo:hi]
        # [h, w, c] -> [w, c, h] -> [WT, 128, CM, HPS]
        negu = (
            np.ascontiguousarray(sl.transpose(1, 2, 0))
            .astype(np.float16)
            .reshape(WT, 128, CM, HPS)
        )
        in_maps.append(
            {
                "negu": negu,
                "bw": bw,
                "bh": build_bh(core, alpha),
                "ident": ident,
            }
        )
    return in_maps, alpha


def gather_output(results):
    """results[core]["qout"]: [WT, 128, C, SH] fp16 -> [H, W, C] fp32."""
    out = np.empty((H, W, C), dtype=np.float32)
    for core in range(NCORES):
        q = results[core]["qout"].astype(np.float32)  # [WT,128,C,SH]
        q = q.reshape(W, C, SH).transpose(2, 0, 1)    # [SH, W, C]
        out[core * SH:(core + 1) * SH] = q
    return out


# ---------------- device kernel ----------------
def seg_split(lo, hi, step=512):
    """Split [lo,hi) at multiples of step."""
    segs = []
    a = lo
    while a < hi:
        b = min(hi, (a // step + 1) * step)
        segs.append((a, b))
        a = b
    return segs


# engine per pass1 PSUM->SBUF copy, index = c * NT + hc (9 entries).
# GPSIMD/Pool cannot read PSUM, so only ACT ("sc") and DVE ("ve").
COPY_ENGINES = ["sc", "ve", "sc", "ve", "sc", "ve", "sc", "ve", "sc"]
# reciprocal engine per w-tile: "ve" = DVE (+1 fuse then InstReciprocal),
# "sc" = ACT ln(s+1) -> exp(-x) (same activation table set as Exp/Copy).
R_ENGINES = ["sc"] * WT


def build_nc(iters=ITERS, repeat=1):
    install_compat()
    nc = bass.Bass("TRN2", target_bir_lowering=False)
    negu_d = nc.dram_tensor("negu", [WT, 128, CM, HPS], F16, kind="ExternalInput")
    bw_d = nc.dram_tensor("bw", [WT, 128, WINP], F16, kind="ExternalInput")
    bh_d = nc.dram_tensor("bh", [NT, 128, WINP], F16, kind="ExternalInput")
    id_d = nc.dram_tensor("ident", [128, 128], F16, kind="ExternalInput")
    qout_d = nc.dram_tensor("qout", [WT, 128, C, SH], F16, kind="ExternalOutput")

    wwins = w_windows()
    hwins = h_windows()

    with tile.TileContext(nc) as tc, ExitStack() as ctx:
        ctx.enter_context(
            nc.allow_low_precision(
                reason="fp16 state by design; softmax internals are fp32"
            )
        )
        pers = ctx.enter_context(tc.tile_pool(name="pers", bufs=1))
        ps_pool = ctx.enter_context(tc.tile_pool(name="ps", bufs=2, space="PSUM"))
        scr = ctx.enter_context(tc.tile_pool(name="scr", bufs=3))
        outp = ctx.enter_context(tc.tile_pool(name="outp", bufs=3))

        negu = []
        qb = []
        for wt in range(WT):
            t = pers.tile([128, CM, HPS], F16, tag=f"negu{wt}", name=f"negu{wt}")
            nc.sync.dma_start(t[:, :, :], negu_d[wt])
            negu.append(t)
            q = pers.tile([128, CM, HPS], F16, tag=f"qb{wt}", name=f"qb{wt}")
            nc.vector.memset(q[:, :, HP:HPS], 0.0)
            qb.append(q)
        spa = [
            [
                pers.tile([128, W], F16, tag=f"spa{hc}_{c}", name=f"spa{hc}_{c}")
                for c in range(CM)
            ]
            for hc in range(NT)
        ]
        bw = []
        for wt in range(WT):
            t = pers.tile([128, WINP], F16, tag=f"bw{wt}", name=f"bwt{wt}")
            nc.sync.dma_start(t[:, :], bw_d[wt])
            bw.append(t)
        bh = []
        for hc in range(NT):
            t = pers.tile([128, WINP], F16, tag=f"bh{hc}", name=f"bht{hc}")
            nc.sync.dma_start(t[:, :], bh_d[hc])
            bh.append(t)
        ident = pers.tile([128, 128], F16, tag="ident", name="ident")
        nc.sync.dma_start(ident[:, :], id_d[:, :])

        def softmax_block(wt, e_src_emit, last, vlo=0, vhi=HP):
            """e_src_emit(e, vlo, vhi) emits the exp instruction into e
            (fp32).  Only rows [vlo, vhi) are computed (validity shrinks
            with the halo each iteration).  e'_3 == 1 implicitly:
            s = 1 + e0 + e1 + e2;  D_c = (e_c - 1) / s."""
            n = vhi - vlo
            e = scr.tile([128, CM, HP], F32, tag="e", name="e")
            e_src_emit(e, vlo, vhi)
            s2 = scr.tile([128, HP], F32, tag="s2", name="s2")
            nc.gpsimd.tensor_tensor(
                out=s2[:, vlo:vhi], in0=e[:, 0, vlo:vhi], in1=e[:, 1, vlo:vhi],
                op=ALU.add,
            )
            s = scr.tile([128, HP], F32, tag="s", name="s")
            nc.gpsimd.tensor_tensor(
                out=s[:, vlo:vhi], in0=s2[:, vlo:vhi], in1=e[:, 2, vlo:vhi],
                op=ALU.add,
            )
            r = scr.tile([128, HP], F32, tag="r", name="r")
            if R_ENGINES[wt] == "ve":
                sp1 = scr.tile([128, HP], F32, tag="sp1", name="sp1")
                nc.vector.tensor_scalar_add(sp1[:, vlo:vhi], s[:, vlo:vhi], 1.0)
                nc.vector.reciprocal(r[:, vlo:vhi], sp1[:, vlo:vhi])
            else:
                # r = 1/(s+1) = exp(-ln(s+1)); the +1 rides the Ln bias.
                lns = scr.tile([128, HP], F32, tag="lns", name="lns")
                nc.scalar.activation(
                    lns[:, vlo:vhi], s[:, vlo:vhi], AF.Ln, bias=1.0
                )
                nc.scalar.activation(
                    r[:, vlo:vhi], lns[:, vlo:vhi], AF.Exp, scale=-1.0
                )
            if not last:
                rb = r[:, vlo:vhi].unsqueeze(1).broadcast_to([128, CM, n])
                nc.vector.scalar_tensor_tensor(
                    out=qb[wt][:, :, vlo:vhi], in0=e[:, :, vlo:vhi],
                    scalar=-1.0, in1=rb, op0=ALU.add, op1=ALU.mult,
                )
            else:
                qo = outp.tile([128, C, SH], F16, tag="qo", name="qo")
                rb = r[:, HALO:HALO + SH].unsqueeze(1).broadcast_to(
                    [128, CM, SH]
                )
                nc.vector.tensor_tensor(
                    out=qo[:, 0:CM, :], in0=e[:, :, HALO:HALO + SH], in1=rb,
                    op=ALU.mult,
                )
                nc.gpsimd.tensor_copy(qo[:, CM, :], r[:, HALO:HALO + SH])
                nc.sync.dma_start(qout_d[wt], qo[:, :, :])

        # ---- optional on-device repeat loop (benchmarking only) ----
        loop_cm = tc.For_i(0, repeat, 1) if repeat > 1 else None
        if loop_cm is not None:
            loop_cm.__enter__()

        # ---- init: D0 from softmax of the unary alone ----
        for wt in range(WT):
            def emit_init(e, vlo, vhi, wt=wt):
                nc.scalar.activation(
                    e[:, :, vlo:vhi], negu[wt][:, :, vlo:vhi], AF.Exp
                )
            softmax_block(wt, emit_init, last=False)

        # ---- iterations ----
        for it in range(iters):
            last = it == iters - 1
            shrink = min(R * (it + 1), HALO)
            vlo, vhi = shrink, HP - shrink
            # pass1: W-blur, B -> A. One 4-bank psum tile per (c, hc);
            # c outer so pass2's first class is ready earliest.
            for c in range(CM):
                for hc in range(NT):
                    ps = ps_pool.tile([128, 4, 512], F32, tag="ps", name="ps")
                    mms = []
                    for wtile in range(WT):
                        lo, hi = wwins[wtile]
                        for (a, b) in seg_split(lo, hi):
                            mms.append((wtile, lo, a, b))
                    # start/stop are per 2KB PSUM bank
                    first_in_bank = [True] * 4
                    last_idx = {}
                    for idx, (wtile, lo, a, b) in enumerate(mms):
                        last_idx[a // 512] = idx
                    for idx, (wtile, lo, a, b) in enumerate(mms):
                        bank = a // 512
                        off = a % 512
                        nc.tensor.matmul(
                            ps[:, bank, off:off + b - a],
                            qb[wtile][:, c, 128 * hc:128 * (hc + 1)],
                            bw[wtile][:, a - lo:b - lo],
                            start=first_in_bank[bank],
                            stop=(last_idx[bank] == idx),
                        )
                        first_in_bank[bank] = False
                    eng = COPY_ENGINES[c * NT + hc]
                    if eng == "sc":
                        nc.scalar.copy(spa[hc][c][:, 0:W], ps[:, :, :])
                    elif eng == "ve":
                        nc.vector.tensor_copy(spa[hc][c][:, 0:W], ps[:, :, :])
                    else:
                        nc.gpsimd.tensor_copy(spa[hc][c][:, 0:W], ps[:, :, :])
            # pass2 + softmax, per w-tile. 3 class banks + identity-negu.
            for wt in range(WT):
                ps = ps_pool.tile([128, 4, 512], F32, tag="ps", name="ps2")
                for c in range(CM):
                    first = True
                    for hc in range(NT):
                        lo, hi = hwins[hc]
                        lo2, hi2 = max(lo, vlo), min(hi, vhi)
                        if lo2 >= hi2:
                            continue
                        nc.tensor.matmul(
                            ps[:, c, lo2:hi2],
                            spa[hc][c][:, 128 * wt:128 * (wt + 1)],
                            bh[hc][:, lo2 - lo:hi2 - lo],
                            start=first,
                            stop=False,
                        )
                        first = False
                    nc.tensor.matmul(
                        ps[:, c, vlo:vhi],
                        ident[:, :],
                        negu[wt][:, c, vlo:vhi],
                        start=False,
                        stop=True,
                    )

                def emit_blur(e, vl, vh, p=ps):
                    nc.scalar.activation(
                        e[:, :, vl:vh], p[:, 0:CM, vl:vh], AF.Exp
                    )
                softmax_block(wt, emit_blur, last=last, vlo=vlo, vhi=vhi)

        if loop_cm is not None:
            loop_cm.__exit__(None, None, None)

    split_multi_waits(nc)
    return nc


_NC_CACHE = None


def get_nc():
    global _NC_CACHE
    if _NC_CACHE is None:
        _NC_CACHE = build_nc()
    return _NC_CACHE


def kernel(unary, image, spatial_weights, compatibility_matrix):
    from concourse.bass_utils import run_bass_kernel_spmd

    in_maps, _ = host_prep(unary, spatial_weights, compatibility_matrix)
    nc = get_nc()
    res = run_bass_kernel_spmd(nc, in_maps, core_ids=list(range(NCORES)))
    return gather_output(res.results)


# revision 11
# speedup vs baseline: 1.2505x; 1.0663x over previous
"""CRF layer (dense CRF with Gaussian spatial kernel) on 8 TRN2 cores.

Per-core: row shard (H/8 rows) + 45-row halo, no inter-core comms.
State lives in B-layout [w-partitions, (class, h)] fp16.

Math restructuring vs the naive form (validated in fp64 to 3e-6):
- Equal Potts alphas + sum_c Q_c = 1  =>  only the differences
  D_c = Q_c - Q_3 (c<3) need blurring; class 3 needs no pairwise term
  (softmax shift invariance, shifting all logits by L_3).
- Shift-free softmax: e'_c = exp(L_c - L_3), e'_3 = 1, so
  s = 1 + sum_{c<3} e'_c,  D_c = (e'_c - 1) / s.  fp32 internals.

Each iteration:
  pass1: W-blur of D as data-stationary banded matmuls (B -> A layout)
  pass2: H-blur likewise (A -> B) + unary-diff via identity matmul (PSUM)
  softmax: exp (ACT, fp32), sums (Pool), reciprocal_approx_fast (DVE),
           D = (e-1)*r (DVE).  PSUM->SBUF copies cycle ACT/DVE/Pool.
Normalization (1/sqrt(blur(ones))) is separable and baked into the band
matrices on the host.
"""
import numpy as np
from contextlib import ExitStack

import concourse.bass as bass
import concourse.mybir as mybir
import concourse.tile as tile
from concourse.vector_clock import ScopedClock, VectorClock

F16 = mybir.dt.float16
F32 = mybir.dt.float32
AF = mybir.ActivationFunctionType
ALU = mybir.AluOpType

# ---------------- problem constants ----------------
H = 2048
W = 2048
C = 4
CM = 3           # blurred classes (differences vs class 3)
SIGMA = 3.0
R = 9            # ceil(3*sigma)
ITERS = 5
NCORES = 8
SH = H // NCORES          # 256 rows per core
HALO = ITERS * R          # 45
HP = SH + 2 * HALO        # 346 rows incl halo
HPS = 384                 # padded to 3*128
NT = HPS // 128           # 3 h tiles
WT = W // 128             # 16 w tiles
WINP = 160                # padded band window (<=146 used)

# ---------------- walrus compat (1 sync-wait per instruction) ----------------
_PATCHED = False


def _patch_drain():
    _orig = tile.TileContext._drain_and_barrier

    def _patched(self, tick_clock, wait_clock):
        gc = tick_clock.global_clock
        n = len(gc)
        for p in range(n):
            t = gc[p]
            if t > 0:
                vec = [0] * n
                vec[p] = t
                nop = self.nc.sync.nop()
                wait_clock.add_sem_waits(
                    nop.ins, ScopedClock({None: VectorClock(vec)})
                )
        full = ScopedClock({None: gc})
        for ec in wait_clock.engine_clocks:
            ec.update_past(full)
        _orig(self, tick_clock, wait_clock)

    tile.TileContext._drain_and_barrier = _patched


def install_compat():
    global _PATCHED
    if not _PATCHED:
        _patch_drain()
        _PATCHED = True


def split_multi_waits(nc):
    """Any instruction with >1 sync wait gets wait-only EventSemaphores
    inserted before it on the same engine (engines run in order)."""
    n_split = 0
    for fn in nc.m.functions:
        for bb in fn.blocks:
            insts = list(bb.instructions)
            out = []
            changed = False
            for inst in insts:
                si = inst.sync_info
                waits = list(si.on_wait) if si is not None else []
                if len(waits) > 1:
                    for j, w in enumerate(waits[:-1]):
                        es = mybir.InstEventSemaphore(
                            name=f"{inst.name}-esw{j}", ins=[], outs=[]
                        )
                        es.engine = inst.engine
                        es.sync_info = mybir.SyncInfo(on_wait=[w], on_update=[])
                        out.append(es)
                        n_split += 1
                    inst.sync_info = mybir.SyncInfo(
                        on_wait=[waits[-1]], on_update=list(si.on_update)
                    )
                    changed = True
                out.append(inst)
            if changed:
                bb.instructions = out
    return n_split


# ---------------- host-side band construction ----------------
def gauss_taps():
    x = np.arange(-R, R + 1, dtype=np.float64)
    return np.exp(-0.5 * (x / SIGMA) ** 2)


def norm_vec(n):
    k = gauss_taps()
    v = np.convolve(np.ones(n, dtype=np.float64), k, mode="same")
    return v


def w_windows():
    wins = []
    for t in range(WT):
        lo = max(0, 128 * t - R)
        hi = min(W, 128 * t + 128 + R)
        wins.append((lo, hi))
    return wins


def h_windows():
    wins = []
    for t in range(NT):
        lo = max(0, 128 * t - R)
        hi = min(HP, 128 * t + 128 + R)
        wins.append((lo, hi))
    return wins


def build_bw():
    """W-direction band blocks [WT, 128, WINP] fp16 (shared by all cores).
    bw[t, i, j] = nw[w_in]*k[w_in-w_out]*nw[w_out]."""
    k = gauss_taps()
    nw = 1.0 / np.sqrt(norm_vec(W))
    out = np.zeros((WT, 128, WINP), dtype=np.float64)
    for t, (lo, hi) in enumerate(w_windows()):
        for i in range(128):
            wi = 128 * t + i
            if wi >= W:
                continue
            for j in range(hi - lo):
                wo = lo + j
                d = wi - wo
                if -R <= d <= R:
                    out[t, i, j] = nw[wi] * k[d + R] * nw[wo]
    return out.astype(np.float16)


def build_bh(core, alpha):
    """H-direction band blocks [NT, 128, WINP] fp16, per core (shared by
    the 3 blurred classes).  Baked: Potts scale (-alpha) and the
    global-row norm (zero at padded rows -> exact zero-pad behavior at
    shard edges)."""
    k = gauss_taps()
    nh_g = 1.0 / np.sqrt(norm_vec(H))
    g0 = core * SH - HALO
    nh = np.zeros(HPS, dtype=np.float64)
    for h in range(HP):
        g = g0 + h
        if 0 <= g < H:
            nh[h] = nh_g[g]
    out = np.zeros((NT, 128, WINP), dtype=np.float64)
    for t, (lo, hi) in enumerate(h_windows()):
        for i in range(128):
            hi_in = 128 * t + i
            if hi_in >= HPS:
                continue
            for j in range(hi - lo):
                ho = lo + j
                d = hi_in - ho
                if -R <= d <= R:
                    out[t, i, j] = -alpha * nh[hi_in] * k[d + R] * nh[ho]
    return out.astype(np.float16)


def host_prep(unary, spatial_weights, compatibility_matrix):
    """Returns (in_maps, alpha). in_maps[core] keys: negu, bw, bh, ident."""
    M = np.asarray(spatial_weights, np.float64) @ np.asarray(
        compatibility_matrix, np.float64
    )
    offd = M - np.diag(np.diag(M))
    if np.abs(offd).max() > 1e-5 * max(np.abs(M).max(), 1e-30):
        raise NotImplementedError(
            "non-diagonal combined compatibility not supported"
        )
    alphas = np.diag(M).copy()
    if not np.allclose(alphas, alphas[0], rtol=1e-6, atol=1e-8):
        raise NotImplementedError("unequal Potts alphas not supported")
    alpha = float(alphas[0])

    bw = build_bw()
    ident = np.eye(128, dtype=np.float16)
    un = np.asarray(unary, np.float32)                  # [H, W, C]
    # L_c - L_3 unary part: u_3 - u_c  (c < 3)
    nprime = un[:, :, 3:4] - un[:, :, 0:3]              # [H, W, 3]

    in_maps = []
    for core in range(NCORES):
        g0 = core * SH - HALO
        sl = np.zeros((HPS, W, CM), dtype=np.float32)
        lo = max(0, g0)
        hi = min(H, g0 + HP)
        sl[lo - g0:hi - g0] = nprime[lo:hi]
        # [h, w, c] -> [w, c, h] -> [WT, 128, CM, HPS]
        negu = (
            np.ascontiguousarray(sl.transpose(1, 2, 0))
            .astype(np.float16)
            .reshape(WT, 128, CM, HPS)
        )
        in_maps.append(
            {
                "negu": negu,
                "bw": bw,
                "bh": build_bh(core, alpha),
                "ident": ident,
            }
        )
    return in_maps, alpha


def gather_output(results):
    """results[core]["qout"]: [WT, 128, C, SH] fp16 -> [H, W, C] fp32."""
    out = np.empty((H, W, C), dtype=np.float32)
    for core in range(NCORES):
        q = results[core]["qout"].astype(np.float32)  # [WT,128,C,SH]
        q = q.reshape(W, C, SH).transpose(2, 0, 1)    # [SH, W, C]
        out[core * SH:(core + 1) * SH] = q
    return out


# ---------------- device kernel ----------------
def seg_split(lo, hi, step=512):
    """Split [lo,hi) at multiples of step."""
    segs = []
    a = lo
    while a < hi:
        b = min(hi, (a // step + 1) * step)
        segs.append((a, b))
        a = b
    return segs


# engine per pass1 PSUM->SBUF copy, index = c * NT + hc (9 entries).
# GPSIMD/Pool cannot read PSUM, so only ACT ("sc") and DVE ("ve").
COPY_ENGINES = ["ve", "ve", "ve", "ve", "sc", "ve", "ve", "ve", "ve"]
# reciprocal engine per w-tile: "ve" = DVE (+1 fuse then InstReciprocal),
# "sc" = ACT ln(s+1) -> exp(-x) (same activation table set as Exp/Copy).
R_ENGINES = ["sc"] * WT


def build_nc(iters=ITERS, repeat=1):
    install_compat()
    nc = bass.Bass("TRN2", target_bir_lowering=False)
    negu_d = nc.dram_tensor("negu", [WT, 128, CM, HPS], F16, kind="ExternalInput")
    bw_d = nc.dram_tensor("bw", [WT, 128, WINP], F16, kind="ExternalInput")
    bh_d = nc.dram_tensor("bh", [NT, 128, WINP], F16, kind="ExternalInput")
    id_d = nc.dram_tensor("ident", [128, 128], F16, kind="ExternalInput")
    qout_d = nc.dram_tensor("qout", [WT, 128, C, SH], F16, kind="ExternalOutput")

    wwins = w_windows()
    hwins = h_windows()

    with tile.TileContext(nc) as tc, ExitStack() as ctx:
        ctx.enter_context(
            nc.allow_low_precision(
                reason="fp16 state by design; softmax internals are fp32"
            )
        )
        pers = ctx.enter_context(tc.tile_pool(name="pers", bufs=1))
        ps_pool = ctx.enter_context(tc.tile_pool(name="ps", bufs=2, space="PSUM"))
        scr = ctx.enter_context(tc.tile_pool(name="scr", bufs=4))
        outp = ctx.enter_context(tc.tile_pool(name="outp", bufs=3))

        negu = []
        qb = []
        for wt in range(WT):
            t = pers.tile([128, CM, HPS], F16, tag=f"negu{wt}", name=f"negu{wt}")
            nc.sync.dma_start(t[:, :, :], negu_d[wt])
            negu.append(t)
            q = pers.tile([128, CM, HPS], F16, tag=f"qb{wt}", name=f"qb{wt}")
            nc.vector.memset(q[:, :, HP:HPS], 0.0)
            qb.append(q)
        spa = [
            [
                pers.tile([128, W], F16, tag=f"spa{hc}_{c}", name=f"spa{hc}_{c}")
                for c in range(CM)
            ]
            for hc in range(NT)
        ]
        bw = []
        for wt in range(WT):
            t = pers.tile([128, WINP], F16, tag=f"bw{wt}", name=f"bwt{wt}")
            nc.sync.dma_start(t[:, :], bw_d[wt])
            bw.append(t)
        bh = []
        for hc in range(NT):
            t = pers.tile([128, WINP], F16, tag=f"bh{hc}", name=f"bht{hc}")
            nc.sync.dma_start(t[:, :], bh_d[hc])
            bh.append(t)
        ident = pers.tile([128, 128], F16, tag="ident", name="ident")
        nc.sync.dma_start(ident[:, :], id_d[:, :])

        def softmax_stages(wt, e_src_emit, last, vlo=0, vhi=HP):
            """4 stage closures for one wt block: S0 exp, S1 sums (Pool),
            S2 reciprocal, S3 D-writeback/output.  e'_3 == 1 implicitly:
            s = 1 + e0 + e1 + e2;  D_c = (e_c - 1) / s."""
            n = vhi - vlo
            st = {}

            def s0():
                e = scr.tile([128, CM, HP], F32, tag="e", name="e")
                st["e"] = e
                e_src_emit(e, vlo, vhi)

            def s1():
                e = st["e"]
                s2 = scr.tile([128, HP], F32, tag="s2", name="s2")
                nc.gpsimd.tensor_tensor(
                    out=s2[:, vlo:vhi], in0=e[:, 0, vlo:vhi],
                    in1=e[:, 1, vlo:vhi], op=ALU.add,
                )
                s = scr.tile([128, HP], F32, tag="s", name="s")
                nc.gpsimd.tensor_tensor(
                    out=s[:, vlo:vhi], in0=s2[:, vlo:vhi],
                    in1=e[:, 2, vlo:vhi], op=ALU.add,
                )
                st["s"] = s

            def s2_():
                s = st["s"]
                r = scr.tile([128, HP], F32, tag="r", name="r")
                if R_ENGINES[wt] == "ve":
                    sp1 = scr.tile([128, HP], F32, tag="sp1", name="sp1")
                    nc.vector.tensor_scalar_add(
                        sp1[:, vlo:vhi], s[:, vlo:vhi], 1.0
                    )
                    nc.vector.reciprocal(r[:, vlo:vhi], sp1[:, vlo:vhi])
                else:
                    # r = 1/(s+1) = exp(-ln(s+1)); the +1 rides the Ln bias.
                    lns = scr.tile([128, HP], F32, tag="lns", name="lns")
                    nc.scalar.activation(
                        lns[:, vlo:vhi], s[:, vlo:vhi], AF.Ln, bias=1.0
                    )
                    nc.scalar.activation(
                        r[:, vlo:vhi], lns[:, vlo:vhi], AF.Exp, scale=-1.0
                    )
                st["r"] = r

            def s3():
                e, r = st["e"], st["r"]
                if not last:
                    rb = r[:, vlo:vhi].unsqueeze(1).broadcast_to([128, CM, n])
                    nc.vector.scalar_tensor_tensor(
                        out=qb[wt][:, :, vlo:vhi], in0=e[:, :, vlo:vhi],
                        scalar=-1.0, in1=rb, op0=ALU.add, op1=ALU.mult,
                    )
                else:
                    qo = outp.tile([128, C, SH], F16, tag="qo", name="qo")
                    rb = r[:, HALO:HALO + SH].unsqueeze(1).broadcast_to(
                        [128, CM, SH]
                    )
                    nc.vector.tensor_tensor(
                        out=qo[:, 0:CM, :], in0=e[:, :, HALO:HALO + SH],
                        in1=rb, op=ALU.mult,
                    )
                    nc.gpsimd.tensor_copy(qo[:, CM, :], r[:, HALO:HALO + SH])
                    nc.sync.dma_start(qout_d[wt], qo[:, :, :])

            return [s0, s1, s2_, s3]

        def run_pipeline(stage_lists):
            """Software-pipelined emission: at slot t emit S3(t-3),
            S2(t-2), S1(t-1), S0(t) — deepest first so each in-order
            engine sees ready work before freshly-gated work."""
            ns = 4
            nblk = len(stage_lists)
            for t in range(nblk + ns - 1):
                for s in reversed(range(ns)):
                    i = t - s
                    if 0 <= i < nblk:
                        stage_lists[i][s]()

        # ---- optional on-device repeat loop (benchmarking only) ----
        loop_cm = tc.For_i(0, repeat, 1) if repeat > 1 else None
        if loop_cm is not None:
            loop_cm.__enter__()

        # ---- init: D0 from softmax of the unary alone ----
        init_stages = []
        for wt in range(WT):
            def emit_init(e, vlo, vhi, wt=wt):
                nc.scalar.activation(
                    e[:, :, vlo:vhi], negu[wt][:, :, vlo:vhi], AF.Exp
                )
            init_stages.append(softmax_stages(wt, emit_init, last=False))
        run_pipeline(init_stages)

        # ---- iterations ----
        for it in range(iters):
            last = it == iters - 1
            shrink = min(R * (it + 1), HALO)
            vlo, vhi = shrink, HP - shrink
            # pass1: W-blur, B -> A. One 4-bank psum tile per (c, hc);
            # c outer so pass2's first class is ready earliest.
            for c in range(CM):
                for hc in range(NT):
                    ps = ps_pool.tile([128, 4, 512], F32, tag="ps", name="ps")
                    mms = []
                    for wtile in range(WT):
                        lo, hi = wwins[wtile]
                        for (a, b) in seg_split(lo, hi):
                            mms.append((wtile, lo, a, b))
                    # start/stop are per 2KB PSUM bank
                    first_in_bank = [True] * 4
                    last_idx = {}
                    for idx, (wtile, lo, a, b) in enumerate(mms):
                        last_idx[a // 512] = idx
                    for idx, (wtile, lo, a, b) in enumerate(mms):
                        bank = a // 512
                        off = a % 512
                        nc.tensor.matmul(
                            ps[:, bank, off:off + b - a],
                            qb[wtile][:, c, 128 * hc:128 * (hc + 1)],
                            bw[wtile][:, a - lo:b - lo],
                            start=first_in_bank[bank],
                            stop=(last_idx[bank] == idx),
                        )
                        first_in_bank[bank] = False
                    eng = COPY_ENGINES[c * NT + hc]
                    if eng == "sc":
                        nc.scalar.copy(spa[hc][c][:, 0:W], ps[:, :, :])
                    elif eng == "ve":
                        nc.vector.tensor_copy(spa[hc][c][:, 0:W], ps[:, :, :])
                    else:
                        nc.gpsimd.tensor_copy(spa[hc][c][:, 0:W], ps[:, :, :])
            # pass2 + softmax, per w-tile. 3 class banks + identity-negu.
            p2_stages = []
            for wt in range(WT):
                def emit_blur(e, vl, vh, wt=wt):
                    ps = ps_pool.tile([128, 4, 512], F32, tag="ps", name="ps2")
                    for c in range(CM):
                        first = True
                        for hc in range(NT):
                            lo, hi = hwins[hc]
                            lo2, hi2 = max(lo, vl), min(hi, vh)
                            if lo2 >= hi2:
                                continue
                            nc.tensor.matmul(
                                ps[:, c, lo2:hi2],
                                spa[hc][c][:, 128 * wt:128 * (wt + 1)],
                                bh[hc][:, lo2 - lo:hi2 - lo],
                                start=first,
                                stop=False,
                            )
                            first = False
                        nc.tensor.matmul(
                            ps[:, c, vl:vh],
                            ident[:, :],
                            negu[wt][:, c, vl:vh],
                            start=False,
                            stop=True,
                        )
                    nc.scalar.activation(
                        e[:, :, vl:vh], ps[:, 0:CM, vl:vh], AF.Exp
                    )
                p2_stages.append(
                    softmax_stages(wt, emit_blur, last=last, vlo=vlo, vhi=vhi)
                )
            run_pipeline(p2_stages)

        if loop_cm is not None:
            loop_cm.__exit__(None, None, None)

    split_multi_waits(nc)
    return nc


_NC_CACHE = None


def get_nc():
    global _NC_CACHE
    if _NC_CACHE is None:
        _NC_CACHE = build_nc()
    return _NC_CACHE


def kernel(unary, image, spatial_weights, compatibility_matrix):
    from concourse.bass_utils import run_bass_kernel_spmd

    in_maps, _ = host_prep(unary, spatial_weights, compatibility_matrix)
    nc = get_nc()
    res = run_bass_kernel_spmd(nc, in_maps, core_ids=list(range(NCORES)))
    return gather_output(res.results)
